# revision 25
# baseline (speedup 1.0000x reference)
"""Fused multi-head causal attention (dense transformer block) on 8 TRN2 NeuronCores.

Sharding: tensor-parallel over heads (16 heads -> 2 heads/core) for the QKV
projection and attention; an on-chip AllToAll then redistributes attention
outputs token-wise so the output projection runs reduce-free (each core
produces the final output rows for its 1/8 token chunk).

All matmuls run as float32r (full PE rate at N>=256, near-fp32 accuracy).
Attention is computed in the transposed orientation S^T = K @ Q^T ([k, q]),
which feeds both the QK and AV matmuls natively.  V is augmented with a
ones-column so softmax denominators fall out of the AV matmul (row 64 of the
[65, q] output).  Causality: upper tiles are skipped entirely; diagonal
128x128 blocks are masked with one host-supplied triangular 0/1 mask.
"""

import math
import sys

import numpy as np

for _p in ("/opt/trn_rl_repo",):
    if _p not in sys.path:
        sys.path.insert(0, _p)

import concourse.bacc as bacc
import concourse.bass as bass
import concourse.mybir as mybir
import concourse.tile as tile
from contextlib import ExitStack

F32 = mybir.dt.float32
F32R = mybir.dt.float32r
EXP = mybir.ActivationFunctionType.Exp

N_CORES = 8
C = 1024
H = 16
DH = 64
DL = (H // N_CORES) * DH  # per-core head-pair dims = 128
QC = 512  # q-chunk width (one PSUM bank of fp32)
KTW = 128  # k-tile width (one partition block)
NCT = C // 128  # contraction tiles for C
AV_LAG = 2  # software-pipeline depth between S-matmuls and AV-matmuls


def build(B, T, n_cores=N_CORES, dbg=False, no_cc=False, phase=4):
    TOK = B * T
    CHUNK = TOK // n_cores
    NJ = T // QC  # q-chunks per batch
    NKT = T // KTW  # k-tiles per batch
    TH = T // 2  # half-batch width for x streaming
    SCALE = 1.0 / math.sqrt(DH)

    nc = bacc.Bacc(
        "TRN2",
        target_bir_lowering=False,
        debug=False,
        enable_asserts=True,
        num_devices=n_cores,
    )
    xT = nc.dram_tensor("xT", [C, TOK], F32R, kind="ExternalInput").ap()
    wqkv = nc.dram_tensor("wqkv", [C, 3 * DL], F32R, kind="ExternalInput").ap()
    wproj = nc.dram_tensor("wproj", [C, C], F32R, kind="ExternalInput").ap()
    tri = nc.dram_tensor("tri", [128, 128], F32R, kind="ExternalInput").ap()
    ident = nc.dram_tensor("ident", [128, 128], F32, kind="ExternalInput").ap()
    ones = nc.dram_tensor("ones", [128, max(T // KTW, 64)], F32R, kind="ExternalInput").ap()
    out = nc.dram_tensor("out", [CHUNK, C], F32, kind="ExternalOutput").ap()
    NKT0 = T // KTW
    dbg_t = {}
    if dbg:
        for dn, shape in [
            ("dbg_qt", [128, T]), ("dbg_kt", [128, T]), ("dbg_va", [128, NKT0 * 65]),
            ("dbg_p", [128, QC]), ("dbg_yst", [65, QC]),
            ("dbg_yn", [64, QC]), ("dbg_a2a", [n_cores, DL, CHUNK]),
            ("dbg_a2ain", [n_cores, DL, CHUNK]),
        ]:
            dbg_t[dn] = nc.dram_tensor(dn, shape, F32, kind="ExternalOutput").ap()

    with tile.TileContext(nc) as tc:
        with ExitStack() as top:
            dram = top.enter_context(tc.tile_pool(name="dram", bufs=1, space="DRAM"))
            a2a_in = dram.tile([n_cores, DL, CHUNK], F32R, name="a2a_in")
            a2a_out = dram.tile([n_cores, DL, CHUNK], F32R, name="a2a_out")

            cpool = top.enter_context(tc.tile_pool(name="consts", bufs=1))
            tri_sb = cpool.tile([128, 128], F32R, name="tri_sb")
            nc.sync.dma_start(tri_sb[:], tri)
            id_sb = cpool.tile([128, 128], F32, name="id_sb")
            nc.sync.dma_start(id_sb[:], ident)
            ones_sb = cpool.tile([128, 64], F32R, name="ones_sb")
            nc.sync.dma_start(ones_sb[:], ones[:, 0:64])
            wq_t = []
            for ct in range(NCT):
                w = cpool.tile([128, 3 * DL], F32R, name=f"wq_{ct}", tag=f"wq_{ct}")
                nc.sync.dma_start(w[:], wqkv[128 * ct : 128 * (ct + 1), :])
                wq_t.append(w)

            with ExitStack() as bsec:
                sb = bsec.enter_context(tc.tile_pool(name="sb", bufs=2))
                ps = bsec.enter_context(tc.tile_pool(name="ps", bufs=2, space="PSUM"))

                for b in range(B):
                    t0 = b * T
                    # ---------------- QKV projection (Q^T, K^T, V^T) -------
                    qt_b = sb.tile([128, T], F32R, name=f"qt_{b}", tag="qt")
                    kt_b = sb.tile([128, T], F32R, name=f"kt_{b}", tag="kt")
                    vaA = sb.tile([128, NKT * 65], F32R, name=f"vaA_{b}", tag="vaugA")
                    vaB = sb.tile([128, NKT * 65], F32R, name=f"vaB_{b}", tag="vaugB")
                    for va in (vaA, vaB):
                        nc.vector.tensor_copy(
                            va.rearrange("p (n s) -> p n s", s=65)[:, :, 64:65],
                            ones_sb[:, 0:NKT].rearrange("p (n s) -> p n s", s=1),
                        )
                    for half in range(2):
                        xt_tiles = []
                        for ct in range(NCT):
                            xt = sb.tile(
                                [128, TH], F32R, name=f"xt_{b}_{half}_{ct}", tag="xt",
                                bufs=2 * NCT,
                            )
                            nc.sync.dma_start(
                                xt[:],
                                xT[128 * ct : 128 * (ct + 1),
                                   t0 + half * TH : t0 + (half + 1) * TH],
                            )
                            xt_tiles.append(xt)
                        vt_h = sb.tile([128, TH], F32, name=f"vt_{b}_{half}", tag="vt")
                        for m in range(3):
                            for j4 in range(TH // QC):
                                acc = ps.tile([128, QC], F32, name=f"qkv_ps_{b}_{half}_{m}_{j4}", tag="mm")
                                for ct in range(NCT):
                                    nc.tensor.matmul(
                                        acc[:],
                                        lhsT=wq_t[ct][:, m * 128 : (m + 1) * 128],
                                        rhs=xt_tiles[ct][:, j4 * QC : (j4 + 1) * QC],
                                        start=(ct == 0),
                                        stop=(ct == NCT - 1),
                                    )
                                dst = half * TH + j4 * QC
                                if m == 0:
                                    nc.vector.tensor_copy(qt_b[:, dst : dst + QC], acc[:])
                                elif m == 1:
                                    nc.vector.tensor_copy(kt_b[:, dst : dst + QC], acc[:])
                                else:
                                    nc.vector.tensor_copy(vt_h[:, j4 * QC : (j4 + 1) * QC], acc[:])
                        # V natural layout via PE transpose; split heads into
                        # the ones-augmented [128, 65] slots.
                        for w in range(TH // 128):
                            ktile = half * (TH // 128) + w
                            vps = ps.tile([128, 128], F32, name=f"vps_{b}_{half}_{w}", tag="mm")
                            nc.tensor.transpose(vps[:], vt_h[:, w * 128 : (w + 1) * 128], id_sb[:])
                            nc.vector.tensor_copy(vaA[:, ktile * 65 : ktile * 65 + 64], vps[:, 0:64])
                            nc.vector.tensor_copy(vaB[:, ktile * 65 : ktile * 65 + 64], vps[:, 64:128])

                    if dbg and b == 0:
                        nc.sync.dma_start(dbg_t["dbg_qt"], qt_b[:].bitcast(F32))
                        nc.sync.dma_start(dbg_t["dbg_kt"], kt_b[:].bitcast(F32))
                        nc.sync.dma_start(dbg_t["dbg_va"], vaA[:].bitcast(F32))

                    if phase < 2:
                        continue
                    # ---------------- attention ---------------------------
                    # Denominator rows live at partitions {0,32,64,96} of
                    # [97, QC] tiles (the only legal partition_broadcast
                    # sources); row r=2j+h -> group r//4, partition 32*(r%4).
                    NG = (2 * NJ + 3) // 4
                    den_g = []
                    for g in range(NG):
                        dg = sb.tile([97, QC], F32, name=f"den_{b}_{g}", tag="den", bufs=NG + 1)
                        nc.vector.memset(dg[:], 1.0)
                        den_g.append(dg)
                    ysts = {}
                    for j in range(NJ):
                        nkt_j = 4 * j + 4
                        yA = ps.tile([65, QC], F32, name=f"yA_{b}_{j}", tag="yaugA", bufs=1)
                        yB = ps.tile([65, QC], F32, name=f"yB_{b}_{j}", tag="yaugB", bufs=1)

                        def emit_av(item):
                            kt_i, c0, pA, pB = item
                            first = kt_i == 0
                            last = kt_i == nkt_j - 1
                            nc.tensor.matmul(
                                yA[:, c0:QC], lhsT=vaA[:, kt_i * 65 : kt_i * 65 + 65],
                                rhs=pA[:, c0:QC], start=first, stop=last,
                            )
                            nc.tensor.matmul(
                                yB[:, c0:QC], lhsT=vaB[:, kt_i * 65 : kt_i * 65 + 65],
                                rhs=pB[:, c0:QC], start=first, stop=last,
                            )

                        pending = []
                        for kt_i in range(nkt_j):
                            psA = ps.tile([128, QC], F32, name=f"sA_{b}_{j}_{kt_i}", tag="psS", bufs=4)
                            psB = ps.tile([128, QC], F32, name=f"sB_{b}_{j}_{kt_i}", tag="psS", bufs=4)
                            nc.tensor.matmul(
                                psA[:], lhsT=kt_b[0:64, kt_i * 128 : (kt_i + 1) * 128],
                                rhs=qt_b[0:64, j * QC : (j + 1) * QC],
                                start=True, stop=True, tile_position=(0, 0),
                            )
                            nc.tensor.matmul(
                                psB[:], lhsT=kt_b[64:128, kt_i * 128 : (kt_i + 1) * 128],
                                rhs=qt_b[64:128, j * QC : (j + 1) * QC],
                                start=True, stop=True, tile_position=(64, 0),
                            )
                            i = kt_i - 4 * j
                            c0 = 128 * i if i >= 0 else 0
                            pA = sb.tile([128, QC], F32R, name=f"pA_{b}_{j}_{kt_i}", tag="pX", bufs=6)
                            pB = sb.tile([128, QC], F32R, name=f"pB_{b}_{j}_{kt_i}", tag="pX", bufs=6)
                            nc.scalar.activation(pA[:, c0:QC], psA[:, c0:QC], EXP, scale=SCALE)
                            nc.scalar.activation(pB[:, c0:QC], psB[:, c0:QC], EXP, scale=SCALE)
                            if i >= 0:
                                nc.vector.tensor_mul(pA[:, c0 : c0 + 128], pA[:, c0 : c0 + 128], tri_sb[:])
                                nc.vector.tensor_mul(pB[:, c0 : c0 + 128], pB[:, c0 : c0 + 128], tri_sb[:])
                            if dbg and b == 0 and j == 0 and kt_i == 0:
                                nc.sync.dma_start(dbg_t["dbg_p"], pA[:].bitcast(F32))
                            pending.append((kt_i, c0, pA, pB))
                            if len(pending) > AV_LAG:
                                emit_av(pending.pop(0))
                        while pending:
                            emit_av(pending.pop(0))

                        # stage y^T + denominators out of PSUM
                        ystA = sb.tile([65, QC], F32, name=f"ystA_{b}_{j}", tag="yst", bufs=2 * NJ + 1)
                        ystB = sb.tile([65, QC], F32, name=f"ystB_{b}_{j}", tag="yst", bufs=2 * NJ + 1)
                        nc.vector.tensor_copy(ystA[:], yA[:])
                        nc.vector.tensor_copy(ystB[:], yB[:])
                        for h, yst in enumerate((ystA, ystB)):
                            r_ = 2 * j + h
                            p0 = 32 * (r_ % 4)
                            nc.sync.dma_start(
                                den_g[r_ // 4][p0 : p0 + 1, :], yst[64:65, :]
                            )
                        if dbg and b == 0 and j == 0:
                            nc.sync.dma_start(dbg_t["dbg_yst"], ystA[:])
                        ysts[j] = (ystA, ystB)

                    # ---------------- normalize + scatter to A2A buffer ----
                    if phase < 3:
                        continue
                    rec_g = []
                    for g in range(NG):
                        rg = sb.tile([97, QC], F32R, name=f"rec_{b}_{g}", tag="rec", bufs=NG + 1)
                        with nc.allow_low_precision(reason="recip rows feed fp32r outer-product"):
                            nc.vector.reciprocal(rg[:], den_g[g][:])
                        rec_g.append(rg)
                    for j in range(NJ):
                        for h in range(2):
                            yst = ysts[j][h]
                            r_ = 2 * j + h
                            p0 = 32 * (r_ % 4)
                            rb = ps.tile([64, QC], F32, name=f"rb_{b}_{j}_{h}", tag="mm")
                            nc.tensor.matmul(
                                rb[:],
                                lhsT=ones_sb[p0 : p0 + 1, :],
                                rhs=rec_g[r_ // 4][p0 : p0 + 1, :],
                                start=True, stop=True,
                                tile_position=(p0, 0),
                            )
                            yn = sb.tile([64, QC], F32R, name=f"yn_{b}_{j}_{h}", tag="yn", bufs=4)
                            nc.vector.tensor_mul(yn[:], yst[0:64, :], rb[:])
                            if dbg and b == 0 and j == 0 and h == 0:
                                nc.sync.dma_start(dbg_t["dbg_yn"], yn[:].bitcast(F32))
                            g = b * T + j * QC
                            width = min(QC, CHUNK)
                            for p in range(QC // width):
                                d = g // CHUNK + (p * width) // CHUNK
                                off = (g + p * width) % CHUNK
                                nc.sync.dma_start(
                                    a2a_in[d, h * 64 : (h + 1) * 64, off : off + width],
                                    yn[:, p * width : (p + 1) * width],
                                )

            # ---------------- AllToAll + output projection ----------------
            if phase < 4:
                no_cc = True
            if not no_cc:
                nc.gpsimd.collective_compute(
                    "AllToAll",
                    mybir.AluOpType.bypass,
                    replica_groups=[list(range(n_cores))],
                    ins=[a2a_in.opt()],
                    outs=[a2a_out.opt()],
                )
            if dbg:
                nc.sync.dma_start(dbg_t["dbg_a2ain"], a2a_in[:].bitcast(F32))
                nc.sync.dma_start(dbg_t["dbg_a2a"], a2a_out[:].bitcast(F32))
            with ExitStack() as psec:
                sb2 = psec.enter_context(tc.tile_pool(name="sb2", bufs=1))
                ps2 = psec.enter_context(tc.tile_pool(name="ps2", bufs=4, space="PSUM"))
                wp_t = []
                for ct in range(NCT):
                    w = sb2.tile([128, C], F32R, name=f"wp_{ct}", tag=f"wp_{ct}")
                    nc.sync.dma_start(w[:], wproj[128 * ct : 128 * (ct + 1), :])
                    wp_t.append(w)
                yt_t = []
                for hh in range(n_cores):
                    yt = sb2.tile([128, CHUNK], F32R, name=f"yt_{hh}", tag=f"yt_{hh}")
                    nc.sync.dma_start(yt[:], a2a_out[hh])
                    yt_t.append(yt)
                for tt in range(CHUNK // 128):
                    for nh in range(C // QC):
                        acc = ps2.tile([128, QC], F32, name=f"o_ps_{tt}_{nh}", tag="mm2")
                        for ct in range(NCT):
                            nc.tensor.matmul(
                                acc[:],
                                lhsT=yt_t[ct][:, tt * 128 : (tt + 1) * 128],
                                rhs=wp_t[ct][:, nh * QC : (nh + 1) * QC],
                                start=(ct == 0),
                                stop=(ct == NCT - 1),
                            )
                        osb = sb2.tile([128, QC], F32, name=f"osb_{tt}_{nh}", tag="osb", bufs=4)
                        nc.vector.tensor_copy(osb[:], acc[:])
                        nc.sync.dma_start(
                            out[tt * 128 : (tt + 1) * 128, nh * QC : (nh + 1) * QC],
                            osb[:],
                        )

    nc.compile()
    return nc


def host_inputs(x, w_attn, w_proj, n_cores=N_CORES):
    B, T, C_ = x.shape
    assert C_ == C
    xT = np.ascontiguousarray(x.reshape(B * T, C).T).astype(np.float32)
    tri_mask = np.triu(np.ones((128, 128), dtype=np.float32))
    identity = np.eye(128, dtype=np.float32)
    in_maps = []
    for c in range(n_cores):
        d0 = c * DL  # first head-pair dim (also head index 2c * 64)
        cols = np.concatenate(
            [np.arange(d0, d0 + DL) + k * C for k in range(3)]
        )
        wq = np.ascontiguousarray(w_attn[:, cols]).astype(np.float32)
        in_maps.append(
            {
                "xT": xT,
                "wqkv": wq,
                "wproj": np.ascontiguousarray(w_proj).astype(np.float32),
                "tri": tri_mask,
                "ident": identity,
                "ones": np.ones((128, max(T // 128, 64)), dtype=np.float32),
            }
        )
    return in_maps


_CACHE = {}


def _get_nc(B, T):
    key = (B, T)
    if key not in _CACHE:
        _CACHE[key] = build(B, T)
    return _CACHE[key]


def run(x, w_attn, w_proj, **spmd_kwargs):
    from concourse.bass_utils import run_bass_kernel_spmd

    x = np.asarray(x, dtype=np.float32)
    B, T, C_ = x.shape
    nc = _get_nc(B, T)
    in_maps = host_inputs(x, np.asarray(w_attn, np.float32), np.asarray(w_proj, np.float32))
    res = run_bass_kernel_spmd(nc, in_maps, core_ids=list(range(N_CORES)), **spmd_kwargs)
    y = np.concatenate([r_["out"] for r_ in res.results], axis=0).reshape(B, T, C_)
    return y, res


def kernel(x, w_attn, b_attn=None, w_proj=None, b_proj=None):
    y, _ = run(x, w_attn, w_proj)
    return y.astype(np.float32)
